# revision 7
# baseline (speedup 1.0000x reference)
import os
import sys
import tempfile

sys.path.insert(0, "/opt/trn_rl_repo")

import numpy as np
import ml_dtypes

import concourse.bacc as bacc
import concourse.mybir as mybir
import concourse.tile as tile
from concourse.bass_utils import run_bass_kernel_spmd

f32 = mybir.dt.float32
bf16 = mybir.dt.bfloat16
AF = mybir.ActivationFunctionType
ALU = mybir.AluOpType
AX = mybir.AxisListType

# Problem dims (hardcoded per contract)
R, B, F, C, NCLS = 32, 4096, 256, 4, 1000
KK, PAD = 5, 1
L0, L1 = 254, 127          # conv1 out, pool1 out
NCORE = 8
BL = B // NCORE            # 512 batch per core
NH = NCLS // 2             # 500: cls half per expert pass

# conv2 j2-blocks
SZ = [13, 13, 13, 13, 10]
JB0 = [0, 13, 26, 39, 52]                    # j2 block starts
BAND = []                                    # l1 band per block
for jb in range(5):
    lo = max(0, 26 * jb - 1)
    hi = min(126, 26 * jb + 2 * SZ[jb] + 2)
    BAND.append((lo, hi - lo + 1))
KJB = [4 * n for _, n in BAND]               # [116,120,120,120,96]
MJB = [8 * s for s in SZ]                    # [104,104,104,104,80]
W1COLS = [4 * n for _, n in BAND for _ in (0, 1)]  # per (jb,e) tile


def _conv1_np(x, w):
    # x: [N, F], w: [C,1,KK] -> [N, C, L0] with pad=1
    xp = np.pad(x, ((0, 0), (PAD, PAD)))
    out = np.zeros((x.shape[0], C, L0), np.float32)
    for c in range(C):
        for k in range(KK):
            out[:, c, :] += w[c, 0, k] * xp[:, k:k + L0]
    return out


def _build_host(proto, c1w, c1b, c2w, c2b, fc1w, fc1b, fc2w):
    bf = ml_dtypes.bfloat16
    # W1: dense conv1 matrix [F, sum(W1COLS)] in (jb,e) tile column order,
    # within tile col = l1loc*4 + c, conv output position (c, l0=2*l1+e)
    tot = sum(W1COLS)
    W1 = np.zeros((F, tot), np.float32)
    off = 0
    for jb in range(5):
        b0, bl = BAND[jb]
        for e in (0, 1):
            for l1loc in range(bl):
                l0 = 2 * (b0 + l1loc) + e
                for c in range(C):
                    col = off + l1loc * 4 + c
                    for k in range(KK):
                        f = l0 + k - 1
                        if 0 <= f < F:
                            W1[f, col] += c1w[c, 0, k]
            off += 4 * bl
    # Q: per-partition scalars [128, R*10] bf16; col = r*10 + (jb*2+e)
    c1p = _conv1_np(proto, c1w)  # [R, C, L0]
    Q = np.zeros((128, R * 10), np.float32)
    for r in range(R):
        t = 0
        for jb in range(5):
            b0, bl = BAND[jb]
            for e in (0, 1):
                for l1loc in range(bl):
                    l0 = 2 * (b0 + l1loc) + e
                    for c in range(C):
                        Q[l1loc * 4 + c, r * 10 + t] = c1b[c] - c1p[r, c, l0]
                t += 1
    # W2B: banded conv2 [128, 5*128] bf16; block jb at free offset jb*128,
    # rows (l1loc, ci), cols (e2, j2loc, co); includes 0.5 pool1 scale
    W2B = np.zeros((128, 5 * 128), np.float32)
    for jb in range(5):
        b0, bl = BAND[jb]
        for e2 in (0, 1):
            for j2loc in range(SZ[jb]):
                l2 = 26 * jb + 2 * j2loc + e2
                for co in range(C):
                    col = e2 * 4 * SZ[jb] + j2loc * 4 + co
                    for kk in range(KK):
                        l1 = l2 - 1 + kk
                        if b0 <= l1 < b0 + bl:
                            for ci in range(C):
                                W2B[(l1 - b0) * 4 + ci, jb * 128 + col] += (
                                    0.5 * c2w[co, ci, kk])
    # B2V: relu2 bias [128, 5]
    B2V = np.zeros((128, 5), np.float32)
    for jb in range(5):
        for e2 in (0, 1):
            for j2loc in range(SZ[jb]):
                for co in range(C):
                    B2V[e2 * 4 * SZ[jb] + j2loc * 4 + co, jb] = c2b[co]
    # FC1W: [128, 5*124] bf16; block jb rows (e2,j2loc,co) -> 0.5*fc1w[co*62+j2]
    FC1W = np.zeros((128, 5 * 124), np.float32)
    for jb in range(5):
        for e2 in (0, 1):
            for j2loc in range(SZ[jb]):
                j2 = JB0[jb] + j2loc
                for co in range(C):
                    FC1W[e2 * 4 * SZ[jb] + j2loc * 4 + co,
                         jb * 124:(jb + 1) * 124] = 0.5 * fc1w[co * 62 + j2, :]
    FC1B = np.zeros((128, 1), np.float32)
    FC1B[:124, 0] = fc1b
    FC2W = np.zeros((128, 1), np.float32)
    FC2W[:124, 0] = fc2w[:, 0]
    ONES = np.ones((32, 1), np.float32)
    return (W1.astype(bf), Q, W2B.astype(bf), B2V,
            FC1W.astype(bf), FC1B, FC2W.astype(bf), ONES.astype(bf))


def _build_program():
    nc = bacc.Bacc("TRN2", target_bir_lowering=False, debug=False,
                   num_devices=NCORE)
    TOT1 = sum(W1COLS)
    dTb_e = nc.declare_dram_parameter("dTb", [F, BL], bf16, isOutput=False)
    W1_e = nc.declare_dram_parameter("W1", [F, TOT1], bf16, isOutput=False)
    Q_e = nc.declare_dram_parameter("Q", [128, R * 10], f32, isOutput=False)
    W2B_e = nc.declare_dram_parameter("W2B", [128, 5 * 128], bf16, isOutput=False)
    B2V_e = nc.declare_dram_parameter("B2V", [128, 5], f32, isOutput=False)
    FC1W_e = nc.declare_dram_parameter("FC1W", [128, 5 * 124], bf16, isOutput=False)
    FC1B_e = nc.declare_dram_parameter("FC1B", [128, 1], f32, isOutput=False)
    FC2W_e = nc.declare_dram_parameter("FC2W", [128, 1], bf16, isOutput=False)
    FC2B_e = nc.declare_dram_parameter("FC2B", [1, 1], f32, isOutput=False)
    ONES_e = nc.declare_dram_parameter("ONES", [32, 1], bf16, isOutput=False)
    CB_e = nc.declare_dram_parameter("CB", [R, NCLS], bf16, isOutput=False)
    CW_e = nc.declare_dram_parameter("CW", [R, F, NCLS], bf16, isOutput=False)
    OUT_e = nc.declare_dram_parameter("OUT", [BL, NCLS], f32, isOutput=True)

    # tile column offsets of W1 per (jb,e)
    w1off = np.cumsum([0] + W1COLS[:-1])

    with tile.TileContext(nc) as tc:
        with (
            tc.tile_pool(name="const", bufs=1) as cp,
            tc.tile_pool(name="work", bufs=4) as wp,
            tc.tile_pool(name="wt", bufs=4) as wtp,
        ):
            dTb = [cp.tile([128, BL], bf16, tag=f"dTb{k}", name=f"dTb{k}")
                   for k in range(2)]
            W1 = [cp.tile([128, TOT1], bf16, tag=f"W1{k}", name=f"W1t{k}")
                  for k in range(2)]
            Qs = cp.tile([128, R * 10], f32, tag="Qs")
            W2B = cp.tile([128, 5 * 128], bf16, tag="W2B")
            B2V = cp.tile([128, 5], f32, tag="B2V")
            FC1W = cp.tile([128, 5 * 124], bf16, tag="FC1W")
            FC1B = cp.tile([128, 1], f32, tag="FC1B")
            FC2W = cp.tile([128, 1], bf16, tag="FC2W")
            FC2B = cp.tile([1, 1], f32, tag="FC2B")
            ONES = cp.tile([32, 1], bf16, tag="ONES")
            CBs = cp.tile([R, NCLS], bf16, tag="CBs")
            fsi = cp.tile([R, BL], bf16, tag="fsi")
            eTr = cp.tile([R, BL], bf16, tag="eTr")
            recipRow = cp.tile([1, BL], f32, tag="recipRow")
            recipT = cp.tile([128, 4], f32, tag="recipT")
            c1d = [cp.tile([128, BL], bf16, tag=f"c1d{t}", name=f"c1d{t}")
                   for t in range(10)]
            sdS = [[cp.tile([128, BL], bf16, tag=f"sd{r}_{k}",
                            name=f"sd{r}_{k}") for k in range(2)]
                   for r in range(R)]

            for k in range(2):
                nc.sync.dma_start(dTb[k][:], dTb_e[k * 128:(k + 1) * 128, :])
                nc.sync.dma_start(W1[k][:], W1_e[k * 128:(k + 1) * 128, :])
            nc.sync.dma_start(Qs[:], Q_e[:])
            nc.sync.dma_start(W2B[:], W2B_e[:])
            nc.sync.dma_start(B2V[:], B2V_e[:])
            nc.sync.dma_start(FC1W[:], FC1W_e[:])
            nc.sync.dma_start(FC1B[:], FC1B_e[:])
            nc.sync.dma_start(FC2W[:], FC2W_e[:])
            nc.sync.dma_start(FC2B[:], FC2B_e[:])
            nc.sync.dma_start(ONES[:], ONES_e[:])
            nc.sync.dma_start(CBs[:], CB_e[:])

            # wt tile prefetch bookkeeping: (rule, k, half) -> tile
            wt_pend = {}

            def wt_dma(r, k, half):
                t = wtp.tile([128, NH], bf16, tag=f"wt{half}_{k}",
                             name=f"wt{half}_{r}_{k}")
                nc.sync.dma_start(
                    t[:], CW_e[r, k * 128:(k + 1) * 128,
                               half * NH:(half + 1) * NH])
                wt_pend[(r, k, half)] = t

            # ---- G1: conv1-dense matmuls -> c1d tiles (bf16) ----
            with tc.tile_pool(name="g1", bufs=2, space="PSUM") as gA:
                for t in range(10):
                    ncol = W1COLS[t]
                    off = int(w1off[t])
                    pg = gA.tile([128, BL], f32, tag="pg", name=f"pg{t}")
                    for k in range(2):
                        nc.tensor.matmul(
                            pg[0:ncol, :], W1[k][:, off:off + ncol], dTb[k][:],
                            start=(k == 0), stop=(k == 1))
                    nc.scalar.activation(c1d[t][0:ncol, :], pg[0:ncol, :],
                                         AF.Copy, bias=0.0, scale=1.0)

            # prefetch expert weights (first pass, cls half 0) for rules 0,1
            for r0 in range(2):
                for k in range(2):
                    wt_dma(r0, k, 0)

            with tc.tile_pool(name="epsB", bufs=1, space="PSUM") as gB:
                eps = [gB.tile([128, NH], f32, tag=f"eps{m}", name=f"eps{m}")
                       for m in range(4)]

                def expert_half(rr, k):
                    wtk = wt_pend.pop((rr, k, 0))
                    for m in range(4):
                        nc.tensor.matmul(
                            eps[m][:],
                            sdS[rr][k][:, m * 128:(m + 1) * 128],
                            wtk[:], start=(rr == 0 and k == 0), stop=False)

                with tc.tile_pool(name="g2", bufs=1, space="PSUM") as gC:
                    zb_prev = None
                    for r in range(R):
                        # vector: relu1 pairs for rule r
                        rls = []
                        for jb in range(5):
                            kj = KJB[jb]
                            rl0 = wp.tile([128, BL], bf16, tag="rl0",
                                          name=f"rl0_{r}_{jb}")
                            rl1 = wp.tile([128, BL], bf16, tag="rl1",
                                          name=f"rl1_{r}_{jb}")
                            c0 = r * 10 + 2 * jb
                            nc.vector.tensor_scalar(
                                rl0[0:kj, :], c1d[2 * jb][0:kj, :],
                                Qs[0:kj, c0:c0 + 1], 0.0, ALU.add, ALU.max)
                            nc.vector.tensor_scalar(
                                rl1[0:kj, :], c1d[2 * jb + 1][0:kj, :],
                                Qs[0:kj, c0 + 1:c0 + 2], 0.0, ALU.add, ALU.max)
                            rls.append((rl0, rl1))

                        # PE: expert k=0 of rule r-2 (always-ready filler)
                        if r >= 2:
                            expert_half(r - 2, 0)

                        # PE: fc2 of rule r-1; scalar: tanh/exp; gpsimd: bcast+sd
                        if r >= 1:
                            psf = gC.tile([1, BL], f32, tag="psf",
                                          name=f"psf{r - 1}")
                            nc.tensor.matmul(psf[0:1, :], FC2W[0:124, 0:1],
                                             zb_prev[0:124, :],
                                             start=True, stop=True)
                            fst = wp.tile([1, BL], bf16, tag="fst",
                                          name=f"fst{r - 1}")
                            nc.scalar.activation(fst[:], psf[0:1, :],
                                                 AF.Tanh, bias=FC2B[0:1, :],
                                                 scale=1.0)
                            nc.sync.dma_start(fsi[r - 1:r, :], fst[:])
                            eR = wp.tile([1, BL], bf16, tag="eR",
                                         name=f"eR{r - 1}")
                            nc.scalar.activation(eR[:], fst[:],
                                                 AF.Exp, bias=0.0, scale=1.0)
                            ebc = wp.tile([128, BL], bf16, tag="ebc",
                                          name=f"ebc{r - 1}")
                            nc.gpsimd.partition_broadcast(ebc[:], eR[0:1, :])
                            for k in range(2):
                                nc.gpsimd.tensor_tensor(
                                    sdS[r - 1][k][:], dTb[k][:], ebc[:],
                                    ALU.mult)

                        # PE: conv2 pairs; relu2 on scalar (jb<3) / gpsimd
                        zps = []
                        for jb in range(5):
                            kj, mj = KJB[jb], MJB[jb]
                            ps2 = gC.tile([128, BL], f32, tag=f"ps2_{jb % 2}",
                                          name=f"ps2_{r}_{jb}")
                            nc.tensor.matmul(
                                ps2[0:mj, :],
                                W2B[0:kj, jb * 128:jb * 128 + mj],
                                rls[jb][0][0:kj, :], start=True, stop=False)
                            nc.tensor.matmul(
                                ps2[0:mj, :],
                                W2B[0:kj, jb * 128:jb * 128 + mj],
                                rls[jb][1][0:kj, :], start=False, stop=True)
                            zp = wp.tile([128, BL], bf16, tag=f"zp{jb}",
                                         name=f"zp_{r}_{jb}")
                            if jb < 4:
                                nc.scalar.activation(
                                    zp[0:mj, :], ps2[0:mj, :], AF.Relu,
                                    bias=B2V[0:mj, jb:jb + 1], scale=1.0)
                            else:
                                nc.vector.tensor_scalar(
                                    zp[0:mj, :], ps2[0:mj, :],
                                    B2V[0:mj, jb:jb + 1], 0.0,
                                    ALU.add, ALU.max)
                            zps.append(zp)

                        # PE: expert k=1 of rule r-2
                        if r >= 2:
                            expert_half(r - 2, 1)

                        # PE: fc1 accumulation
                        psz = gC.tile([128, BL], f32, tag="psz",
                                      name=f"psz{r}")
                        for jb in range(5):
                            mj = MJB[jb]
                            nc.tensor.matmul(
                                psz[0:124, :],
                                FC1W[0:mj, jb * 124:(jb + 1) * 124],
                                zps[jb][0:mj, :],
                                start=(jb == 0), stop=(jb == 4))
                        zb = wp.tile([128, BL], bf16, tag="zb",
                                     name=f"zb{r}")
                        nc.scalar.activation(zb[0:124, :], psz[0:124, :],
                                             AF.Relu, bias=FC1B[0:124, :],
                                             scale=1.0)
                        zb_prev = zb

                        # prefetch pass-1 wt two rules ahead
                        if r + 2 < R:
                            for k in range(2):
                                wt_dma(r + 2, k, 0)
                        # prefetch pass-2 (cls half 1) wt late in phase B
                        if r >= 26:
                            rr = (r - 26) * 2
                            for k in range(2):
                                wt_dma(rr, k, 1)
                                wt_dma(rr + 1, k, 1)

                    # ---- tail gating for rule 31 ----
                    psf = gC.tile([1, BL], f32, tag="psf", name="psf31")
                    nc.tensor.matmul(psf[0:1, :], FC2W[0:124, 0:1],
                                     zb_prev[0:124, :], start=True, stop=True)
                    fst = wp.tile([1, BL], bf16, tag="fst", name="fst31")
                    nc.scalar.activation(fst[:], psf[0:1, :],
                                         AF.Tanh, bias=FC2B[0:1, :], scale=1.0)
                    nc.sync.dma_start(fsi[R - 1:R, :], fst[:])
                    eR = wp.tile([1, BL], bf16, tag="eR", name="eR31")
                    nc.scalar.activation(eR[:], fst[:], AF.Exp,
                                         bias=0.0, scale=1.0)
                    ebc = wp.tile([128, BL], bf16, tag="ebc", name="ebc31")
                    nc.gpsimd.partition_broadcast(ebc[:], eR[0:1, :])
                    for k in range(2):
                        nc.gpsimd.tensor_tensor(sdS[R - 1][k][:], dTb[k][:],
                                                ebc[:], ALU.mult)
                    for rr in (R - 2, R - 1):
                        expert_half(rr, 0)
                        expert_half(rr, 1)

                    # softmax denominator: ssum = 1^T exp(fsi); recip
                    nc.scalar.activation(eTr[:], fsi[:], AF.Exp, bias=0.0,
                                         scale=1.0)
                    pss = gC.tile([1, BL], f32, tag="psf", name="pss")
                    nc.tensor.matmul(pss[0:1, :], ONES[0:32, 0:1],
                                     eTr[0:32, :], start=True, stop=True)
                    nc.vector.reciprocal(recipRow[:], pss[0:1, :])
                    for m in range(4):
                        nc.sync.dma_start(recipT[0:128, m:m + 1],
                                          recipRow[0:1, m * 128:(m + 1) * 128])

                # ---- drain cls half 0: bias matmul + scale + store ----
                for m in range(4):
                    nc.tensor.matmul(
                        eps[m][:], eTr[:, m * 128:(m + 1) * 128],
                        CBs[:, 0:NH], start=False, stop=True)
                    osb = wp.tile([128, NH], f32, tag="osb",
                                  name=f"osb0_{m}")
                    nc.scalar.activation(osb[:], eps[m][:], AF.Copy,
                                         bias=0.0, scale=recipT[:, m:m + 1])
                    nc.sync.dma_start(
                        OUT_e[m * 128:(m + 1) * 128, 0:NH], osb[:])

                # ---- expert pass 2: cls half 1, dense PE tail ----
                with tc.tile_pool(name="epsC", bufs=1, space="PSUM") as gD:
                    eps2 = [gD.tile([128, NH], f32, tag=f"eps2{m}",
                                    name=f"eps2{m}") for m in range(4)]
                    for r in range(R):
                        for k in range(2):
                            wtk = wt_pend.pop((r, k, 1))
                            for m in range(4):
                                nc.tensor.matmul(
                                    eps2[m][:],
                                    sdS[r][k][:, m * 128:(m + 1) * 128],
                                    wtk[:], start=(r == 0 and k == 0),
                                    stop=False)
                        rr = r + 12
                        if rr < R:
                            for k in range(2):
                                wt_dma(rr, k, 1)
                    for m in range(4):
                        nc.tensor.matmul(
                            eps2[m][:], eTr[:, m * 128:(m + 1) * 128],
                            CBs[:, NH:2 * NH], start=False, stop=True)
                        osb = wp.tile([128, NH], f32, tag="osb",
                                      name=f"osb1_{m}")
                        nc.scalar.activation(osb[:], eps2[m][:], AF.Copy,
                                             bias=0.0,
                                             scale=recipT[:, m:m + 1])
                        nc.sync.dma_start(
                            OUT_e[m * 128:(m + 1) * 128, NH:2 * NH], osb[:])
    nc.compile()
    return nc


_CACHE = {}


def kernel(data, proto, conv1_w, conv1_b, conv2_w, conv2_b,
           fc1_w, fc1_b, fc2_w, fc2_b, consq_w, consq_b, is_train=0,
           trace=False, tmpdir=None):
    bf = ml_dtypes.bfloat16
    data = np.asarray(data, np.float32)
    (W1, Q, W2B, B2V, FC1W, FC1B, FC2W, ONES) = _build_host(
        np.asarray(proto, np.float32), np.asarray(conv1_w, np.float32),
        np.asarray(conv1_b, np.float32), np.asarray(conv2_w, np.float32),
        np.asarray(conv2_b, np.float32), np.asarray(fc1_w, np.float32),
        np.asarray(fc1_b, np.float32), np.asarray(fc2_w, np.float32))
    if "nc" not in _CACHE:
        _CACHE["nc"] = _build_program()
    nc = _CACHE["nc"]

    CW = np.ascontiguousarray(
        np.asarray(consq_w, np.float32).astype(ml_dtypes.bfloat16))
    CB = np.ascontiguousarray(
        np.asarray(consq_b, np.float32).astype(ml_dtypes.bfloat16))
    FC2B = np.array([[np.asarray(fc2_b, np.float32).reshape(-1)[0]]],
                    np.float32)
    shared = dict(W1=np.ascontiguousarray(W1), Q=np.ascontiguousarray(Q),
                  W2B=np.ascontiguousarray(W2B), B2V=np.ascontiguousarray(B2V),
                  FC1W=np.ascontiguousarray(FC1W),
                  FC1B=np.ascontiguousarray(FC1B),
                  FC2W=np.ascontiguousarray(FC2W), FC2B=FC2B,
                  ONES=np.ascontiguousarray(ONES), CB=CB, CW=CW)
    in_maps = []
    for i in range(NCORE):
        dsl = data[i * BL:(i + 1) * BL, :]
        dTi = np.ascontiguousarray(dsl.T.astype(bf))
        in_maps.append(dict(shared, dTb=dTi))
    res = run_bass_kernel_spmd(
        nc, in_maps, list(range(NCORE)), trace=trace,
        tmpdir=tmpdir or (tempfile.mkdtemp(prefix="moek_") if trace else None))
    out = np.concatenate([res.results[i]["OUT"] for i in range(NCORE)], axis=0)
    kernel.last_exec_time_ns = res.exec_time_ns
    return out
